# revision 22
# baseline (speedup 1.0000x reference)
"""Two-layer LSTM (H=51) over [B=4096, T=256] on 8 NeuronCores.

Strategy: data-parallel over batch (512 per core). Per core, a skewed
software pipeline over T+2 phases: phase q computes layer-1 of step q,
layer-2 of step q-1, and the linear head of step q-2.

Key structure per phase per group (two batch groups pipeline against
each other):
  - 4 fused gate matmuls (banks f,i,o,g), each combining layer-1 (output
    rows 0..50) and layer-2 (rows 64..114) against the shared state
    stack; f32r operands keep the PE sequencer at hardware-decode cost
    (bf16 matmuls would emit 233ns Ldweights sequencer ops).
  - The linear head rides along as lhsT column 115 of the g bank: PSUM
    row 115 = W_lin @ h2(q-2) + b_lin for free; one [1,Bg] copy per
    phase stages it into a [128,Bg] SBUF tile, flushed to a [T,B] DRAM
    tensor every 128 phases (host transposes to [B,T]).
  - x rides in stk rows 116..123: 4 timesteps per DMA, double-buffered
    (8 rotating lhsT variants select the right x row), so the SP
    sequencer issues one x-DMA per 4 phases instead of one per phase.
  - Elementwise: sf,si,so = sigmoid(z_fio) (one ACT op), tg = tanh(z_g);
    u = sf*c, v = si*tg, c' = u+v (bf16 tensor_tensor, 2x DVE mode);
    tc = tanh(c'); h' = so*tc (writes the f32r stk directly).
Phase 0's spurious layer-2 output is cancelled by re-zeroing h2/c2
right after phase 0.
"""

import numpy as np

H = 51
T_FULL = 256
B_FULL = 4096
N_CORES = 8
XBLK = 4  # timesteps per x DMA block (double-buffered -> 8 x rows)

# Stk partition layout (stacked matmul rhs):
#   rows 0..50   : h1
#   rows 51..63  : junk (zero, weighted by zero)
#   rows 64..114 : h2
#   row 115      : ones (bias row, memset to 1)
#   rows 116..123: x for steps 4k..4k+3, double buffered
ROW_H1 = 0
ROW_JUNK = 51
ROW_H2 = 64
ROW_ONES = 115
ROW_X = 116
K_STK = 124
# gate-row space of the elementwise ops: rows 0..50 layer1, 51..63 junk,
# 64..114 layer2
GP = 115

MW = 115  # lhsT width per bank (zero-padded gate columns)
NVAR = 2 * XBLK  # lhsT variants (x row rotates with q mod 8)


def _build_weights(W_ih1, W_hh1, b_ih1, b_hh1, W_ih2, W_hh2, b_ih2, b_hh2,
                   W_lin, b_lin):
    """Host-side packing of lhsT weight tiles.

    Returns WG [K_STK, NVAR*4*MW + 1] float64: 8 variants (q mod 8
    selects which x row feeds layer-1) x 4 banks (f,i,o,g) x width 115,
    plus the out-head column: W_lin at h2 rows, b_lin at the ones row
    (its lhsT is stk[64:116] = [h2; ones]).
    """
    b1 = (b_ih1 + b_hh1).astype(np.float64)
    b2 = (b_ih2 + b_hh2).astype(np.float64)
    # reference gate order in the stacked 4H rows: i, f, g, o
    idx = {"i": np.arange(0, H), "f": np.arange(H, 2 * H),
           "g": np.arange(2 * H, 3 * H), "o": np.arange(3 * H, 4 * H)}
    # banks: f, i, o (sigmoid, contiguous) then g (tanh)
    order = ["f", "i", "o", "g"]
    WG = np.zeros((K_STK, NVAR * 4 * MW + 1), dtype=np.float64)
    for r in range(NVAR):
        for xi, gate in enumerate(order):
            rows = idx[gate]
            c0 = (r * 4 + xi) * MW
            col1 = slice(c0, c0 + H)
            WG[ROW_ONES, col1] = b1[rows]
            WG[ROW_H1:ROW_H1 + H, col1] = W_hh1[rows, :].T
            WG[ROW_X + r, col1] = W_ih1[rows, 0]
            col2 = slice(c0 + ROW_H2, c0 + ROW_H2 + H)
            WG[ROW_ONES, col2] = b2[rows]
            WG[ROW_H1:ROW_H1 + H, col2] = W_ih2[rows, :].T
            WG[ROW_H2:ROW_H2 + H, col2] = W_hh2[rows, :].T
    WG[ROW_H2:ROW_H2 + H, NVAR * 4 * MW] = W_lin[0, :]
    WG[ROW_ONES, NVAR * 4 * MW] = float(np.asarray(b_lin).reshape(-1)[0])
    return WG


def build_core_kernel(T, B, groups=2, mode="bf16"):
    """Build the per-core Bass kernel. Inputs: xT [T+1, B] (row 0 = ones),
    WG [K_STK, 3681]. Output: out_bt [B, T] (full head incl. b_lin)."""
    import concourse.bacc as bacc
    import concourse.mybir as mybir
    from concourse.tile import TileContext

    fp = mybir.dt.float32
    fpr = mybir.dt.float32r
    dt_e = mybir.dt.bfloat16 if mode == "bf16" else fp
    Bg = B // groups
    assert T % XBLK == 0

    nc = bacc.Bacc("TRN2", target_bir_lowering=False, debug=False)
    # xT row 0 is a host-prepended row of ones (feeds the bias row of Stk);
    # rows 1..T are input.T
    xT = nc.dram_tensor("xT", [T + 1, B], fpr, kind="ExternalInput")
    WG = nc.dram_tensor("WG", [K_STK, NVAR * 4 * MW + 1], fpr,
                        kind="ExternalInput")
    out_bt = nc.dram_tensor("out_bt", [B, T], fp, kind="ExternalOutput")

    C = min(128, T)  # output columns buffered in PSUM between flushes
    assert T % C == 0
    assert (B // groups) % 128 == 0

    with TileContext(nc) as tc:
        with (
            tc.tile_pool(name="persist", bufs=1) as persist,
            tc.tile_pool(name="gpsum", bufs=1, space="PSUM") as gpsum,
            tc.tile_pool(name="opsum", bufs=1, space="PSUM") as opsum,
            tc.tile_pool(name="temps", bufs=3) as temps,
            tc.tile_pool(name="ostage", bufs=2) as ostage,
        ):
            wg = persist.tile([K_STK, NVAR * 4 * MW + 1], fpr)
            nc.sync.dma_start(out=wg, in_=WG[:, :])

            nchunk = Bg // 128
            stks, cts, gps, pos = [], [], [], []
            for g in range(groups):
                stk = persist.tile([K_STK, Bg], fpr, tag=f"stk{g}")
                ct = persist.tile([GP, Bg], dt_e, tag=f"ct{g}")
                gp = gpsum.tile([GP, 4 * Bg], fp, tag=f"gp{g}")
                # memset through an f32 view (f32r cannot be memset directly)
                nc.vector.memset(stk[:, :].bitcast(fp), 0.0)
                nc.vector.memset(ct[:, :].bitcast(fp), 0.0)
                # ones row comes from xT row 0 (DMA has no partition
                # alignment constraint, unlike compute-engine writes)
                nc.sync.dma_start(out=stk[ROW_ONES:ROW_ONES + 1, :],
                                  in_=xT[0:1, g * Bg:(g + 1) * Bg])
                # x block 0 (steps 0..3) into rows 116..119
                nc.sync.dma_start(out=stk[ROW_X:ROW_X + XBLK, :],
                                  in_=xT[1:1 + XBLK, g * Bg:(g + 1) * Bg])
                stks.append(stk)
                cts.append(ct)
                gps.append(gp)
                pos.append(opsum.tile([128, nchunk * C], fp, tag=f"po{g}",
                                      name=f"po{g}"))

            mult = mybir.AluOpType.mult
            add = mybir.AluOpType.add
            tanh = mybir.ActivationFunctionType.Tanh
            sigm = mybir.ActivationFunctionType.Sigmoid

            for q in range(T + 2):
                mm = q <= T
                var = q % NVAR
                # ---- x prefetch: at the start of block k, fetch block k+1
                # into the other x-row half (its readers finished 2 blocks
                # ago, so the DMA is fully off the critical path).
                for g in range(groups):
                    stk, gp = stks[g], gps[g]
                    cols = slice(g * Bg, (g + 1) * Bg)
                    if q % XBLK == 0 and q + XBLK < T:
                        k1 = q // XBLK + 1
                        r0 = ROW_X + (k1 % 2) * XBLK
                        nc.sync.dma_start(
                            out=stk[r0:r0 + XBLK, :],
                            in_=xT[1 + k1 * XBLK:1 + (k1 + 1) * XBLK, cols])
                    if mm:
                        rhs = stk[0:K_STK, :]
                        for xi in range(4):
                            cb = (var * 4 + xi) * MW
                            nc.tensor.matmul(
                                gp[0:GP, xi * Bg:(xi + 1) * Bg],
                                wg[0:K_STK, cb:cb + MW],
                                rhs, start=True, stop=True)
                # ---- out head for step t = q-2: out[:, t] column
                if q >= 2:
                    t = q - 2
                    tc_col = t % C
                    hc = NVAR * 4 * MW
                    for g in range(groups):
                        stk = stks[g]
                        for k in range(nchunk):
                            # f32r rejects N=1 matmuls; use f32 bitcast views
                            nc.tensor.matmul(
                                pos[g][:, k * C + tc_col:k * C + tc_col + 1],
                                stk[64:116, k * 128:(k + 1) * 128].bitcast(fp),
                                wg[64:116, hc:hc + 1].bitcast(fp),
                                start=True, stop=True)
                    if tc_col == C - 1:  # flush epoch
                        t0 = t - (C - 1)
                        for g in range(groups):
                            for k in range(nchunk):
                                st = ostage.tile([128, C], fp, tag=f"os{g}_{k}")
                                nc.vector.tensor_copy(
                                    st, pos[g][:, k * C:(k + 1) * C])
                                row0 = g * Bg + k * 128
                                nc.sync.dma_start(
                                    out=out_bt[row0:row0 + 128, t0:t0 + C],
                                    in_=st)
                # ---- elementwise chain per group. Banks: 0=f, 1=i, 2=o, 3=g.
                if mm:
                    for g in range(groups):
                        sg_t = temps.tile([GP, 4 * Bg], dt_e, tag=f"sg{g}")
                        nc.scalar.activation(sg_t[:, 0:3 * Bg],
                                             gps[g][0:GP, 0:3 * Bg], sigm)
                        nc.scalar.activation(sg_t[:, 3 * Bg:4 * Bg],
                                             gps[g][0:GP, 3 * Bg:4 * Bg], tanh)
                        sf = sg_t[:, 0 * Bg:1 * Bg]
                        si = sg_t[:, 1 * Bg:2 * Bg]
                        so = sg_t[:, 2 * Bg:3 * Bg]
                        tg = sg_t[:, 3 * Bg:4 * Bg]
                        v = temps.tile([GP, Bg], dt_e, tag=f"v{g}")
                        u = temps.tile([GP, Bg], dt_e, tag=f"u{g}")
                        tcl = temps.tile([GP, Bg], dt_e, tag=f"tc{g}")
                        # u = sf*c ; v = si*tg ; c' = u+v
                        nc.vector.tensor_tensor(u, sf, cts[g][:, :], mult)
                        nc.vector.tensor_tensor(v, si, tg, mult)
                        nc.vector.tensor_tensor(cts[g][:, :], u, v, add)
                        # tc = tanh(c') ; h' = so*tc (into the f32r stk)
                        nc.scalar.activation(tcl, cts[g][:, :], tanh)
                        nc.vector.tensor_tensor(
                            stks[g][ROW_H1:ROW_H1 + GP, :], so, tcl, mult)
                if q == 0:
                    # cancel phase 0's spurious l2 output: h2/c2 must enter
                    # phase 1 as zero.
                    for g in range(groups):
                        nc.vector.memset(
                            stks[g][ROW_H2:ROW_H2 + H, :].bitcast(fp), 0.0)
                        nc.vector.memset(
                            cts[g][ROW_H2:ROW_H2 + H, :].bitcast(fp), 0.0)
    nc.compile()
    return nc


_NC_CACHE = {}


def _get_nc(T, B, groups, mode):
    key = (T, B, groups, mode)
    if key not in _NC_CACHE:
        _NC_CACHE[key] = build_core_kernel(T, B, groups, mode)
    return _NC_CACHE[key]


def kernel(input, W_ih1, W_hh1, b_ih1, b_hh1, W_ih2, W_hh2, b_ih2, b_hh2,
           W_lin, b_lin, _groups=2, _mode="bf16"):
    from concourse import bass_utils

    input = np.asarray(input, dtype=np.float32)
    B, T = input.shape
    Bc = B // N_CORES
    WG = _build_weights(np.asarray(W_ih1, np.float64), np.asarray(W_hh1, np.float64),
                        np.asarray(b_ih1, np.float64), np.asarray(b_hh1, np.float64),
                        np.asarray(W_ih2, np.float64), np.asarray(W_hh2, np.float64),
                        np.asarray(b_ih2, np.float64), np.asarray(b_hh2, np.float64),
                        np.asarray(W_lin, np.float64), np.asarray(b_lin, np.float64))
    # row 0 = ones (bias row), rows 1..T = input.T
    xT = np.concatenate([np.ones((1, B), np.float32),
                         input.T.astype(np.float32)])
    WGh = np.ascontiguousarray(WG).astype(np.float32)
    nc = _get_nc(T, Bc, _groups, _mode)
    in_maps = [
        {"xT": np.ascontiguousarray(xT[:, c * Bc:(c + 1) * Bc]), "WG": WGh}
        for c in range(N_CORES)
    ]
    res = bass_utils.run_bass_kernel_spmd(
        nc, in_maps, core_ids=list(range(N_CORES)), trace=False)
    outs = [res.results[c]["out_bt"] for c in range(N_CORES)]  # [Bc, T] each
    out = np.concatenate(outs, axis=0)  # [B, T]
    return out.astype(np.float32)


# revision 23
# speedup vs baseline: 1.0007x; 1.0007x over previous
"""Two-layer LSTM (H=51) over [B=4096, T=256] on 8 NeuronCores.

Strategy: data-parallel over batch (512 per core). Per core, a skewed
software pipeline over T+2 phases: phase q computes layer-1 of step q,
layer-2 of step q-1, and the linear head of step q-2.

Key structure per phase per group (two batch groups pipeline against
each other):
  - 4 fused gate matmuls (banks f,i,o,g), each combining layer-1 (output
    rows 0..50) and layer-2 (rows 64..114) against the shared state
    stack; f32r operands keep the PE sequencer at hardware-decode cost
    (bf16 matmuls would emit 233ns Ldweights sequencer ops).
  - The linear head rides along as lhsT column 115 of the g bank: PSUM
    row 115 = W_lin @ h2(q-2) + b_lin for free; one [1,Bg] copy per
    phase stages it into a [128,Bg] SBUF tile, flushed to a [T,B] DRAM
    tensor every 128 phases (host transposes to [B,T]).
  - x rides in stk rows 116..123: 4 timesteps per DMA, double-buffered
    (8 rotating lhsT variants select the right x row), so the SP
    sequencer issues one x-DMA per 4 phases instead of one per phase.
  - Elementwise: sf,si,so = sigmoid(z_fio) (one ACT op), tg = tanh(z_g);
    u = sf*c, v = si*tg, c' = u+v (bf16 tensor_tensor, 2x DVE mode);
    tc = tanh(c'); h' = so*tc (writes the f32r stk directly).
Phase 0's spurious layer-2 output is cancelled by re-zeroing h2/c2
right after phase 0.
"""

import numpy as np

H = 51
T_FULL = 256
B_FULL = 4096
N_CORES = 8
XBLK = 4  # timesteps per x DMA block (double-buffered -> 8 x rows)

# Stk partition layout (stacked matmul rhs):
#   rows 0..50   : h1
#   rows 51..63  : junk (zero, weighted by zero)
#   rows 64..114 : h2
#   row 115      : ones (bias row, memset to 1)
#   rows 116..123: x for steps 4k..4k+3, double buffered
ROW_H1 = 0
ROW_JUNK = 51
ROW_H2 = 64
ROW_ONES = 115
ROW_X = 116
K_STK = 124
# gate-row space of the elementwise ops: rows 0..50 layer1, 51..63 junk,
# 64..114 layer2
GP = 115

MW = 115  # lhsT width per bank (zero-padded gate columns)
NVAR = 2 * XBLK  # lhsT variants (x row rotates with q mod 8)


def _build_weights(W_ih1, W_hh1, b_ih1, b_hh1, W_ih2, W_hh2, b_ih2, b_hh2,
                   W_lin, b_lin):
    """Host-side packing of lhsT weight tiles.

    Returns WG [K_STK, NVAR*4*MW + 1] float64: 8 variants (q mod 8
    selects which x row feeds layer-1) x 4 banks (f,i,o,g) x width 115,
    plus the out-head column: W_lin at h2 rows, b_lin at the ones row
    (its lhsT is stk[64:116] = [h2; ones]).
    """
    b1 = (b_ih1 + b_hh1).astype(np.float64)
    b2 = (b_ih2 + b_hh2).astype(np.float64)
    # reference gate order in the stacked 4H rows: i, f, g, o
    idx = {"i": np.arange(0, H), "f": np.arange(H, 2 * H),
           "g": np.arange(2 * H, 3 * H), "o": np.arange(3 * H, 4 * H)}
    # banks: f, i, o (sigmoid, contiguous) then g (tanh)
    order = ["f", "i", "o", "g"]
    WG = np.zeros((K_STK, NVAR * 4 * MW + 1), dtype=np.float64)
    for r in range(NVAR):
        for xi, gate in enumerate(order):
            rows = idx[gate]
            c0 = (r * 4 + xi) * MW
            col1 = slice(c0, c0 + H)
            WG[ROW_ONES, col1] = b1[rows]
            WG[ROW_H1:ROW_H1 + H, col1] = W_hh1[rows, :].T
            WG[ROW_X + r, col1] = W_ih1[rows, 0]
            col2 = slice(c0 + ROW_H2, c0 + ROW_H2 + H)
            WG[ROW_ONES, col2] = b2[rows]
            WG[ROW_H1:ROW_H1 + H, col2] = W_ih2[rows, :].T
            WG[ROW_H2:ROW_H2 + H, col2] = W_hh2[rows, :].T
    WG[ROW_H2:ROW_H2 + H, NVAR * 4 * MW] = W_lin[0, :]
    WG[ROW_ONES, NVAR * 4 * MW] = float(np.asarray(b_lin).reshape(-1)[0])
    return WG


def build_core_kernel(T, B, groups=2, mode="bf16"):
    """Build the per-core Bass kernel. Inputs: xT [T+1, B] (row 0 = ones),
    WG [K_STK, 3681]. Output: out_bt [B, T] (full head incl. b_lin)."""
    import concourse.bacc as bacc
    import concourse.mybir as mybir
    from concourse.tile import TileContext

    fp = mybir.dt.float32
    fpr = mybir.dt.float32r
    dt_e = mybir.dt.bfloat16 if mode == "bf16" else fp
    Bg = B // groups
    assert T % XBLK == 0

    nc = bacc.Bacc("TRN2", target_bir_lowering=False, debug=False)
    # xT row 0 is a host-prepended row of ones (feeds the bias row of Stk);
    # rows 1..T are input.T
    xT = nc.dram_tensor("xT", [T + 1, B], fpr, kind="ExternalInput")
    WG = nc.dram_tensor("WG", [K_STK, NVAR * 4 * MW + 1], fpr,
                        kind="ExternalInput")
    out_bt = nc.dram_tensor("out_bt", [B, T], fp, kind="ExternalOutput")

    C = min(128, T)  # output columns buffered in PSUM between flushes
    assert T % C == 0
    assert (B // groups) % 128 == 0

    with TileContext(nc) as tc:
        with (
            tc.tile_pool(name="persist", bufs=1) as persist,
            tc.tile_pool(name="gpsum", bufs=1, space="PSUM") as gpsum,
            tc.tile_pool(name="opsum", bufs=1, space="PSUM") as opsum,
            tc.tile_pool(name="temps", bufs=3) as temps,
            tc.tile_pool(name="ostage", bufs=2) as ostage,
        ):
            wg = persist.tile([K_STK, NVAR * 4 * MW + 1], fpr)
            nc.sync.dma_start(out=wg, in_=WG[:, :])

            nchunk = Bg // 128
            stks, cts, gps, pos = [], [], [], []
            for g in range(groups):
                stk = persist.tile([K_STK, Bg], fpr, tag=f"stk{g}")
                ct = persist.tile([GP, Bg], dt_e, tag=f"ct{g}")
                gp = gpsum.tile([GP, 4 * Bg], fp, tag=f"gp{g}")
                # memset through an f32 view (f32r cannot be memset directly)
                nc.vector.memset(stk[:, :].bitcast(fp), 0.0)
                nc.vector.memset(ct[:, :].bitcast(fp), 0.0)
                # ones row comes from xT row 0 (DMA has no partition
                # alignment constraint, unlike compute-engine writes)
                nc.sync.dma_start(out=stk[ROW_ONES:ROW_ONES + 1, :],
                                  in_=xT[0:1, g * Bg:(g + 1) * Bg])
                # x block 0 (steps 0..3) into rows 116..119
                nc.sync.dma_start(out=stk[ROW_X:ROW_X + XBLK, :],
                                  in_=xT[1:1 + XBLK, g * Bg:(g + 1) * Bg])
                stks.append(stk)
                cts.append(ct)
                gps.append(gp)
                pos.append(opsum.tile([128, nchunk * C], fp, tag=f"po{g}",
                                      name=f"po{g}"))

            mult = mybir.AluOpType.mult
            add = mybir.AluOpType.add
            tanh = mybir.ActivationFunctionType.Tanh
            sigm = mybir.ActivationFunctionType.Sigmoid

            for q in range(T + 1):
                mm = q <= T
                var = q % NVAR
                # ---- x prefetch: at the start of block k, fetch block k+1
                # into the other x-row half (its readers finished 2 blocks
                # ago, so the DMA is fully off the critical path).
                for g in range(groups):
                    stk, gp = stks[g], gps[g]
                    cols = slice(g * Bg, (g + 1) * Bg)
                    if q % XBLK == 0 and q + XBLK < T:
                        k1 = q // XBLK + 1
                        r0 = ROW_X + (k1 % 2) * XBLK
                        nc.sync.dma_start(
                            out=stk[r0:r0 + XBLK, :],
                            in_=xT[1 + k1 * XBLK:1 + (k1 + 1) * XBLK, cols])
                    if mm:
                        rhs = stk[0:K_STK, :]
                        for xi in range(4):
                            cb = (var * 4 + xi) * MW
                            nc.tensor.matmul(
                                gp[0:GP, xi * Bg:(xi + 1) * Bg],
                                wg[0:K_STK, cb:cb + MW],
                                rhs, start=True, stop=True)
                # ---- elementwise chain per group. Banks: 0=f, 1=i, 2=o, 3=g.
                if mm:
                    for g in range(groups):
                        sg_t = temps.tile([GP, 4 * Bg], dt_e, tag=f"sg{g}")
                        nc.scalar.activation(sg_t[:, 0:3 * Bg],
                                             gps[g][0:GP, 0:3 * Bg], sigm)
                        nc.scalar.activation(sg_t[:, 3 * Bg:4 * Bg],
                                             gps[g][0:GP, 3 * Bg:4 * Bg], tanh)
                        sf = sg_t[:, 0 * Bg:1 * Bg]
                        si = sg_t[:, 1 * Bg:2 * Bg]
                        so = sg_t[:, 2 * Bg:3 * Bg]
                        tg = sg_t[:, 3 * Bg:4 * Bg]
                        v = temps.tile([GP, Bg], dt_e, tag=f"v{g}")
                        u = temps.tile([GP, Bg], dt_e, tag=f"u{g}")
                        tcl = temps.tile([GP, Bg], dt_e, tag=f"tc{g}")
                        # u = sf*c ; v = si*tg ; c' = u+v
                        nc.vector.tensor_tensor(u, sf, cts[g][:, :], mult)
                        nc.vector.tensor_tensor(v, si, tg, mult)
                        nc.vector.tensor_tensor(cts[g][:, :], u, v, add)
                        # tc = tanh(c') ; h' = so*tc (into the f32r stk)
                        nc.scalar.activation(tcl, cts[g][:, :], tanh)
                        nc.vector.tensor_tensor(
                            stks[g][ROW_H1:ROW_H1 + GP, :], so, tcl, mult)
                        # ---- out head for step t = q-1 (reads the h2 rows
                        # just written by ht: h2(q-1)). Emitted here so the
                        # head matmuls run mid-phase, which also keeps the PE
                        # from idling long enough to drop out of its fast
                        # pstate.
                        if q >= 1:
                            t = q - 1
                            tc_col = t % C
                            hc = NVAR * 4 * MW
                            for k in range(nchunk):
                                # f32r rejects N=1 matmuls; f32 bitcast views
                                nc.tensor.matmul(
                                    pos[g][:, k * C + tc_col:k * C + tc_col + 1],
                                    stks[g][64:116, k * 128:(k + 1) * 128]
                                    .bitcast(fp),
                                    wg[64:116, hc:hc + 1].bitcast(fp),
                                    start=True, stop=True)
                            if tc_col == C - 1:  # flush epoch
                                t0 = t - (C - 1)
                                for k in range(nchunk):
                                    st = ostage.tile([128, C], fp,
                                                     tag=f"os{g}_{k}",
                                                     name=f"os{g}_{k}")
                                    nc.vector.tensor_copy(
                                        st, pos[g][:, k * C:(k + 1) * C])
                                    row0 = g * Bg + k * 128
                                    nc.sync.dma_start(
                                        out=out_bt[row0:row0 + 128, t0:t0 + C],
                                        in_=st)
                if q == 0:
                    # cancel phase 0's spurious l2 output: h2/c2 must enter
                    # phase 1 as zero.
                    for g in range(groups):
                        nc.vector.memset(
                            stks[g][ROW_H2:ROW_H2 + H, :].bitcast(fp), 0.0)
                        nc.vector.memset(
                            cts[g][ROW_H2:ROW_H2 + H, :].bitcast(fp), 0.0)
    nc.compile()
    return nc


_NC_CACHE = {}


def _get_nc(T, B, groups, mode):
    key = (T, B, groups, mode)
    if key not in _NC_CACHE:
        _NC_CACHE[key] = build_core_kernel(T, B, groups, mode)
    return _NC_CACHE[key]


def kernel(input, W_ih1, W_hh1, b_ih1, b_hh1, W_ih2, W_hh2, b_ih2, b_hh2,
           W_lin, b_lin, _groups=2, _mode="bf16"):
    from concourse import bass_utils

    input = np.asarray(input, dtype=np.float32)
    B, T = input.shape
    Bc = B // N_CORES
    WG = _build_weights(np.asarray(W_ih1, np.float64), np.asarray(W_hh1, np.float64),
                        np.asarray(b_ih1, np.float64), np.asarray(b_hh1, np.float64),
                        np.asarray(W_ih2, np.float64), np.asarray(W_hh2, np.float64),
                        np.asarray(b_ih2, np.float64), np.asarray(b_hh2, np.float64),
                        np.asarray(W_lin, np.float64), np.asarray(b_lin, np.float64))
    # row 0 = ones (bias row), rows 1..T = input.T
    xT = np.concatenate([np.ones((1, B), np.float32),
                         input.T.astype(np.float32)])
    WGh = np.ascontiguousarray(WG).astype(np.float32)
    nc = _get_nc(T, Bc, _groups, _mode)
    in_maps = [
        {"xT": np.ascontiguousarray(xT[:, c * Bc:(c + 1) * Bc]), "WG": WGh}
        for c in range(N_CORES)
    ]
    res = bass_utils.run_bass_kernel_spmd(
        nc, in_maps, core_ids=list(range(N_CORES)), trace=False)
    outs = [res.results[c]["out_bt"] for c in range(N_CORES)]  # [Bc, T] each
    out = np.concatenate(outs, axis=0)  # [B, T]
    return out.astype(np.float32)


# revision 25
# speedup vs baseline: 1.1244x; 1.1236x over previous
"""Two-layer LSTM (H=51) over [B=4096, T=256] on 8 NeuronCores.

Strategy: data-parallel over batch (512 per core). Per core, a skewed
software pipeline over T+2 phases: phase q computes layer-1 of step q,
layer-2 of step q-1, and the linear head of step q-2.

Key structure per phase per group (two batch groups pipeline against
each other):
  - 4 fused gate matmuls (banks f,i,o,g), each combining layer-1 (output
    rows 0..50) and layer-2 (rows 64..114) against the shared state
    stack; f32r operands keep the PE sequencer at hardware-decode cost
    (bf16 matmuls would emit 233ns Ldweights sequencer ops).
  - The linear head rides along as lhsT column 115 of the g bank: PSUM
    row 115 = W_lin @ h2(q-2) + b_lin for free; one [1,Bg] copy per
    phase stages it into a [128,Bg] SBUF tile, flushed to a [T,B] DRAM
    tensor every 128 phases (host transposes to [B,T]).
  - x rides in stk rows 116..123: 4 timesteps per DMA, double-buffered
    (8 rotating lhsT variants select the right x row), so the SP
    sequencer issues one x-DMA per 4 phases instead of one per phase.
  - Elementwise: sf,si,so = sigmoid(z_fio) (one ACT op), tg = tanh(z_g);
    u = sf*c, v = si*tg, c' = u+v (bf16 tensor_tensor, 2x DVE mode);
    tc = tanh(c'); h' = so*tc (writes the f32r stk directly).
Phase 0's spurious layer-2 output is cancelled by re-zeroing h2/c2
right after phase 0.
"""

import numpy as np

H = 51
T_FULL = 256
B_FULL = 4096
N_CORES = 8
XBLK = 4  # timesteps per x DMA block (double-buffered -> 8 x rows)

# Stk partition layout (stacked matmul rhs):
#   rows 0..50   : h1
#   rows 51..63  : junk (zero, weighted by zero)
#   rows 64..114 : h2
#   row 115      : ones (bias row, memset to 1)
#   rows 116..123: x for steps 4k..4k+3, double buffered
ROW_H1 = 0
ROW_JUNK = 51
ROW_H2 = 64
ROW_ONES = 115
ROW_X = 116
K_STK = 124
# gate-row space of the elementwise ops: rows 0..50 layer1, 51..63 junk,
# 64..114 layer2
GP = 115

MW = 115  # lhsT width per bank (zero-padded gate columns)
NVAR = 2 * XBLK  # lhsT variants (x row rotates with q mod 8)


def _build_weights(W_ih1, W_hh1, b_ih1, b_hh1, W_ih2, W_hh2, b_ih2, b_hh2,
                   W_lin, b_lin):
    """Host-side packing of lhsT weight tiles.

    Returns WG [K_STK, NVAR*4*MW + 1] float64: 8 variants (q mod 8
    selects which x row feeds layer-1) x 4 banks (f,i,o,g) x width 115,
    plus the out-head column: W_lin at h2 rows, b_lin at the ones row
    (its lhsT is stk[64:116] = [h2; ones]).
    """
    b1 = (b_ih1 + b_hh1).astype(np.float64)
    b2 = (b_ih2 + b_hh2).astype(np.float64)
    # reference gate order in the stacked 4H rows: i, f, g, o
    idx = {"i": np.arange(0, H), "f": np.arange(H, 2 * H),
           "g": np.arange(2 * H, 3 * H), "o": np.arange(3 * H, 4 * H)}
    # banks: f, i, o (sigmoid, contiguous) then g (tanh)
    order = ["f", "i", "o", "g"]
    WG = np.zeros((K_STK, NVAR * 4 * MW + 1), dtype=np.float64)
    for r in range(NVAR):
        for xi, gate in enumerate(order):
            rows = idx[gate]
            c0 = (r * 4 + xi) * MW
            col1 = slice(c0, c0 + H)
            WG[ROW_ONES, col1] = b1[rows]
            WG[ROW_H1:ROW_H1 + H, col1] = W_hh1[rows, :].T
            WG[ROW_X + r, col1] = W_ih1[rows, 0]
            col2 = slice(c0 + ROW_H2, c0 + ROW_H2 + H)
            WG[ROW_ONES, col2] = b2[rows]
            WG[ROW_H1:ROW_H1 + H, col2] = W_ih2[rows, :].T
            WG[ROW_H2:ROW_H2 + H, col2] = W_hh2[rows, :].T
    WG[ROW_H2:ROW_H2 + H, NVAR * 4 * MW] = W_lin[0, :]
    WG[ROW_ONES, NVAR * 4 * MW] = float(np.asarray(b_lin).reshape(-1)[0])
    return WG


def build_core_kernel(T, B, groups=2, mode="bf16"):
    """Build the per-core Bass kernel. Inputs: xT [T+1, B] (row 0 = ones),
    WG [K_STK, 3681]. Output: out_bt [B, T] (full head incl. b_lin)."""
    import concourse.bacc as bacc
    import concourse.mybir as mybir
    from concourse.tile import TileContext

    fp = mybir.dt.float32
    fpr = mybir.dt.float32r
    dt_e = mybir.dt.bfloat16 if mode == "bf16" else fp
    Bg = B // groups
    assert T % XBLK == 0

    nc = bacc.Bacc("TRN2", target_bir_lowering=False, debug=False)
    # xT row 0 is a host-prepended row of ones (feeds the bias row of Stk);
    # rows 1..T are input.T
    xT = nc.dram_tensor("xT", [T + 1, B], fpr, kind="ExternalInput")
    WG = nc.dram_tensor("WG", [K_STK, NVAR * 4 * MW + 1], fpr,
                        kind="ExternalInput")
    out_bt = nc.dram_tensor("out_bt", [B, T], fp, kind="ExternalOutput")

    C = min(128, T)  # output columns buffered in PSUM between flushes
    assert T % C == 0
    assert (B // groups) % 128 == 0

    with TileContext(nc) as tc:
        with (
            tc.tile_pool(name="persist", bufs=1) as persist,
            tc.tile_pool(name="gpsum", bufs=1, space="PSUM") as gpsum,
            tc.tile_pool(name="opsum", bufs=1, space="PSUM") as opsum,
            tc.tile_pool(name="temps", bufs=3) as temps,
            tc.tile_pool(name="ostage", bufs=2) as ostage,
        ):
            wg = persist.tile([K_STK, NVAR * 4 * MW + 1], fpr)
            nc.sync.dma_start(out=wg, in_=WG[:, :])

            nchunk = Bg // 128
            stks, cts, gps, pos = [], [], [], []
            for g in range(groups):
                stk = persist.tile([K_STK, Bg], fpr, tag=f"stk{g}")
                ct = persist.tile([GP, Bg], dt_e, tag=f"ct{g}")
                gp = gpsum.tile([GP, 4 * Bg], fp, tag=f"gp{g}")
                # memset through an f32 view (f32r cannot be memset directly)
                nc.vector.memset(stk[:, :].bitcast(fp), 0.0)
                nc.vector.memset(ct[:, :].bitcast(fp), 0.0)
                # ones row comes from xT row 0 (DMA has no partition
                # alignment constraint, unlike compute-engine writes)
                nc.sync.dma_start(out=stk[ROW_ONES:ROW_ONES + 1, :],
                                  in_=xT[0:1, g * Bg:(g + 1) * Bg])
                # x block 0 (steps 0..3) into rows 116..119
                nc.sync.dma_start(out=stk[ROW_X:ROW_X + XBLK, :],
                                  in_=xT[1:1 + XBLK, g * Bg:(g + 1) * Bg])
                stks.append(stk)
                cts.append(ct)
                gps.append(gp)
                pos.append(opsum.tile([128, nchunk * C], fp, tag=f"po{g}",
                                      name=f"po{g}"))
            warm = opsum.tile([128, 8], fp, tag="warm", name="warm")

            mult = mybir.AluOpType.mult
            add = mybir.AluOpType.add
            tanh = mybir.ActivationFunctionType.Tanh
            sigm = mybir.ActivationFunctionType.Sigmoid

            for q in range(T + 1):
                mm = q <= T
                var = q % NVAR
                # ---- x prefetch: at the start of block k, fetch block k+1
                # into the other x-row half (its readers finished 2 blocks
                # ago, so the DMA is fully off the critical path).
                for g in range(groups):
                    stk, gp = stks[g], gps[g]
                    cols = slice(g * Bg, (g + 1) * Bg)
                    if q % XBLK == 0 and q + XBLK < T:
                        k1 = q // XBLK + 1
                        r0 = ROW_X + (k1 % 2) * XBLK
                        nc.sync.dma_start(
                            out=stk[r0:r0 + XBLK, :],
                            in_=xT[1 + k1 * XBLK:1 + (k1 + 1) * XBLK, cols])
                    if mm:
                        rhs = stk[0:K_STK, :]
                        for xi in range(4):
                            cb = (var * 4 + xi) * MW
                            nc.tensor.matmul(
                                gp[0:GP, xi * Bg:(xi + 1) * Bg],
                                wg[0:K_STK, cb:cb + MW],
                                rhs, start=True, stop=True)
                # ---- elementwise chain per group. Banks: 0=f, 1=i, 2=o, 3=g.
                if mm:
                    for g in range(groups):
                        sg_t = temps.tile([GP, 4 * Bg], dt_e, tag=f"sg{g}")
                        nc.scalar.activation(sg_t[:, 0:3 * Bg],
                                             gps[g][0:GP, 0:3 * Bg], sigm)
                        nc.scalar.activation(sg_t[:, 3 * Bg:4 * Bg],
                                             gps[g][0:GP, 3 * Bg:4 * Bg], tanh)
                        sf = sg_t[:, 0 * Bg:1 * Bg]
                        si = sg_t[:, 1 * Bg:2 * Bg]
                        so = sg_t[:, 2 * Bg:3 * Bg]
                        tg = sg_t[:, 3 * Bg:4 * Bg]
                        v = temps.tile([GP, Bg], dt_e, tag=f"v{g}")
                        u = temps.tile([GP, Bg], dt_e, tag=f"u{g}")
                        tcl = temps.tile([GP, Bg], dt_e, tag=f"tc{g}")
                        # u = sf*c ; v = si*tg ; c' = u+v
                        nc.vector.tensor_tensor(u, sf, cts[g][:, :], mult)
                        nc.vector.tensor_tensor(v, si, tg, mult)
                        nc.vector.tensor_tensor(cts[g][:, :], u, v, add)
                        # PE keep-warm: a ~7ns f32 N=1 matmul anchored on
                        # sg_t runs mid-phase, so the PE never idles long
                        # enough for the cost model to drop it out of the
                        # fast pstate (which would turn the next phase's
                        # 107ns gate matmuls into 394ns ones).
                        sgf = sg_t.bitcast(fp)
                        nc.tensor.matmul(warm[:, g:g + 1], sgf[0:32, 0:128],
                                         sgf[0:32, 0:1], start=True, stop=True)
                        # tc = tanh(c') ; h' = so*tc (into the f32r stk)
                        nc.scalar.activation(tcl, cts[g][:, :], tanh)
                        nc.vector.tensor_tensor(
                            stks[g][ROW_H1:ROW_H1 + GP, :], so, tcl, mult)
                        tcf = tcl.bitcast(fp)
                        nc.tensor.matmul(warm[0:64, 4 + g:5 + g],
                                         tcf[0:32, 0:64], tcf[0:32, 0:1],
                                         start=True, stop=True)
                        # ---- out head for step t = q-1 (reads the h2 rows
                        # just written by ht: h2(q-1)). Emitted here so the
                        # head matmuls run mid-phase, which also keeps the PE
                        # from idling long enough to drop out of its fast
                        # pstate.
                        if q >= 1:
                            t = q - 1
                            tc_col = t % C
                            hc = NVAR * 4 * MW
                            for k in range(nchunk):
                                # f32r rejects N=1 matmuls; f32 bitcast views
                                nc.tensor.matmul(
                                    pos[g][:, k * C + tc_col:k * C + tc_col + 1],
                                    stks[g][64:116, k * 128:(k + 1) * 128]
                                    .bitcast(fp),
                                    wg[64:116, hc:hc + 1].bitcast(fp),
                                    start=True, stop=True)
                            if tc_col == C - 1:  # flush epoch
                                t0 = t - (C - 1)
                                for k in range(nchunk):
                                    st = ostage.tile([128, C], fp,
                                                     tag=f"os{g}_{k}",
                                                     name=f"os{g}_{k}")
                                    nc.vector.tensor_copy(
                                        st, pos[g][:, k * C:(k + 1) * C])
                                    row0 = g * Bg + k * 128
                                    nc.sync.dma_start(
                                        out=out_bt[row0:row0 + 128, t0:t0 + C],
                                        in_=st)
                if q == 0:
                    # cancel phase 0's spurious l2 output: h2/c2 must enter
                    # phase 1 as zero.
                    for g in range(groups):
                        nc.vector.memset(
                            stks[g][ROW_H2:ROW_H2 + H, :].bitcast(fp), 0.0)
                        nc.vector.memset(
                            cts[g][ROW_H2:ROW_H2 + H, :].bitcast(fp), 0.0)
    nc.compile()
    return nc


_NC_CACHE = {}


def _get_nc(T, B, groups, mode):
    key = (T, B, groups, mode)
    if key not in _NC_CACHE:
        _NC_CACHE[key] = build_core_kernel(T, B, groups, mode)
    return _NC_CACHE[key]


def kernel(input, W_ih1, W_hh1, b_ih1, b_hh1, W_ih2, W_hh2, b_ih2, b_hh2,
           W_lin, b_lin, _groups=2, _mode="bf16"):
    from concourse import bass_utils

    input = np.asarray(input, dtype=np.float32)
    B, T = input.shape
    Bc = B // N_CORES
    WG = _build_weights(np.asarray(W_ih1, np.float64), np.asarray(W_hh1, np.float64),
                        np.asarray(b_ih1, np.float64), np.asarray(b_hh1, np.float64),
                        np.asarray(W_ih2, np.float64), np.asarray(W_hh2, np.float64),
                        np.asarray(b_ih2, np.float64), np.asarray(b_hh2, np.float64),
                        np.asarray(W_lin, np.float64), np.asarray(b_lin, np.float64))
    # row 0 = ones (bias row), rows 1..T = input.T
    xT = np.concatenate([np.ones((1, B), np.float32),
                         input.T.astype(np.float32)])
    WGh = np.ascontiguousarray(WG).astype(np.float32)
    nc = _get_nc(T, Bc, _groups, _mode)
    in_maps = [
        {"xT": np.ascontiguousarray(xT[:, c * Bc:(c + 1) * Bc]), "WG": WGh}
        for c in range(N_CORES)
    ]
    res = bass_utils.run_bass_kernel_spmd(
        nc, in_maps, core_ids=list(range(N_CORES)), trace=False)
    outs = [res.results[c]["out_bt"] for c in range(N_CORES)]  # [Bc, T] each
    out = np.concatenate(outs, axis=0)  # [B, T]
    return out.astype(np.float32)


# revision 26
# speedup vs baseline: 1.1529x; 1.0254x over previous
"""Two-layer LSTM (H=51) over [B=4096, T=256] on 8 NeuronCores.

Strategy: data-parallel over batch (512 per core). Per core, a skewed
software pipeline over T+2 phases: phase q computes layer-1 of step q,
layer-2 of step q-1, and the linear head of step q-2.

Key structure per phase per group (two batch groups pipeline against
each other):
  - 4 fused gate matmuls (banks f,i,o,g), each combining layer-1 (output
    rows 0..50) and layer-2 (rows 64..114) against the shared state
    stack; f32r operands keep the PE sequencer at hardware-decode cost
    (bf16 matmuls would emit 233ns Ldweights sequencer ops).
  - The linear head rides along as lhsT column 115 of the g bank: PSUM
    row 115 = W_lin @ h2(q-2) + b_lin for free; one [1,Bg] copy per
    phase stages it into a [128,Bg] SBUF tile, flushed to a [T,B] DRAM
    tensor every 128 phases (host transposes to [B,T]).
  - x rides in stk rows 116..123: 4 timesteps per DMA, double-buffered
    (8 rotating lhsT variants select the right x row), so the SP
    sequencer issues one x-DMA per 4 phases instead of one per phase.
  - Elementwise: sf,si,so = sigmoid(z_fio) (one ACT op), tg = tanh(z_g);
    u = sf*c, v = si*tg, c' = u+v (bf16 tensor_tensor, 2x DVE mode);
    tc = tanh(c'); h' = so*tc (writes the f32r stk directly).
Phase 0's spurious layer-2 output is cancelled by re-zeroing h2/c2
right after phase 0.
"""

import numpy as np

H = 51
T_FULL = 256
B_FULL = 4096
N_CORES = 8
XBLK = 4  # timesteps per x DMA block (double-buffered -> 8 x rows)

# Stk partition layout (stacked matmul rhs):
#   rows 0..50   : h1
#   rows 51..63  : junk (zero, weighted by zero)
#   rows 64..114 : h2
#   row 115      : ones (bias row, memset to 1)
#   rows 116..123: x for steps 4k..4k+3, double buffered
ROW_H1 = 0
ROW_JUNK = 51
ROW_H2 = 64
ROW_ONES = 115
ROW_X = 116
K_STK = 124
# gate-row space of the elementwise ops: rows 0..50 layer1, 51..63 junk,
# 64..114 layer2
GP = 115

MW = 115  # lhsT width per bank (zero-padded gate columns)
NVAR = 2 * XBLK  # lhsT variants (x row rotates with q mod 8)


def _build_weights(W_ih1, W_hh1, b_ih1, b_hh1, W_ih2, W_hh2, b_ih2, b_hh2,
                   W_lin, b_lin):
    """Host-side packing of lhsT weight tiles.

    Returns WG [K_STK, NVAR*4*MW + 1] float64: 8 variants (q mod 8
    selects which x row feeds layer-1) x 4 banks (f,i,o,g) x width 115,
    plus the out-head column: W_lin at h2 rows, b_lin at the ones row
    (its lhsT is stk[64:116] = [h2; ones]).
    """
    b1 = (b_ih1 + b_hh1).astype(np.float64)
    b2 = (b_ih2 + b_hh2).astype(np.float64)
    # reference gate order in the stacked 4H rows: i, f, g, o
    idx = {"i": np.arange(0, H), "f": np.arange(H, 2 * H),
           "g": np.arange(2 * H, 3 * H), "o": np.arange(3 * H, 4 * H)}
    # banks: f, i, g, o. All sigmoids become tanh(z/2) (scale 0.5 folded
    # into the weights); states stored doubled (ht=2h, ct=2c) so h inputs
    # carry an extra 0.5.
    order = ["f", "i", "g", "o"]
    WG = np.zeros((K_STK, NVAR * 4 * MW + 1), dtype=np.float64)
    for r in range(NVAR):
        for xi, gate in enumerate(order):
            rows = idx[gate]
            sc = 0.5 if gate in ("i", "f", "o") else 1.0
            c0 = (r * 4 + xi) * MW
            col1 = slice(c0, c0 + H)
            WG[ROW_ONES, col1] = sc * b1[rows]
            WG[ROW_H1:ROW_H1 + H, col1] = sc * 0.5 * W_hh1[rows, :].T
            WG[ROW_X + r, col1] = sc * W_ih1[rows, 0]
            col2 = slice(c0 + ROW_H2, c0 + ROW_H2 + H)
            WG[ROW_ONES, col2] = sc * b2[rows]
            WG[ROW_H1:ROW_H1 + H, col2] = sc * 0.5 * W_ih2[rows, :].T
            WG[ROW_H2:ROW_H2 + H, col2] = sc * 0.5 * W_hh2[rows, :].T
    WG[ROW_H2:ROW_H2 + H, NVAR * 4 * MW] = 0.5 * W_lin[0, :]
    WG[ROW_ONES, NVAR * 4 * MW] = float(np.asarray(b_lin).reshape(-1)[0])
    return WG


def build_core_kernel(T, B, groups=2, mode="bf16"):
    """Build the per-core Bass kernel. Inputs: xT [T+1, B] (row 0 = ones),
    WG [K_STK, 3681]. Output: out_bt [B, T] (full head incl. b_lin)."""
    import concourse.bacc as bacc
    import concourse.mybir as mybir
    from concourse.tile import TileContext

    fp = mybir.dt.float32
    fpr = mybir.dt.float32r
    dt_e = fp  # stt combines gain nothing from bf16
    Bg = B // groups
    assert T % XBLK == 0

    nc = bacc.Bacc("TRN2", target_bir_lowering=False, debug=False)
    # xT row 0 is a host-prepended row of ones (feeds the bias row of Stk);
    # rows 1..T are input.T
    xT = nc.dram_tensor("xT", [T + 1, B], fpr, kind="ExternalInput")
    WG = nc.dram_tensor("WG", [K_STK, NVAR * 4 * MW + 1], fpr,
                        kind="ExternalInput")
    out_bt = nc.dram_tensor("out_bt", [B, T], fp, kind="ExternalOutput")

    C = min(128, T)  # output columns buffered in PSUM between flushes
    assert T % C == 0
    assert (B // groups) % 128 == 0

    with TileContext(nc) as tc:
        with (
            tc.tile_pool(name="persist", bufs=1) as persist,
            tc.tile_pool(name="gpsum", bufs=1, space="PSUM") as gpsum,
            tc.tile_pool(name="opsum", bufs=1, space="PSUM") as opsum,
            tc.tile_pool(name="temps", bufs=3) as temps,
            tc.tile_pool(name="ostage", bufs=2) as ostage,
        ):
            wg = persist.tile([K_STK, NVAR * 4 * MW + 1], fpr)
            nc.sync.dma_start(out=wg, in_=WG[:, :])

            nchunk = Bg // 128
            stks, cts, gps, pos = [], [], [], []
            for g in range(groups):
                stk = persist.tile([K_STK, Bg], fpr, tag=f"stk{g}")
                ct = persist.tile([GP, Bg], dt_e, tag=f"ct{g}")
                gp = gpsum.tile([GP, 4 * Bg], fp, tag=f"gp{g}")
                # memset through an f32 view (f32r cannot be memset directly)
                nc.vector.memset(stk[:, :].bitcast(fp), 0.0)
                nc.vector.memset(ct[:, :].bitcast(fp), 0.0)
                # ones row comes from xT row 0 (DMA has no partition
                # alignment constraint, unlike compute-engine writes)
                nc.sync.dma_start(out=stk[ROW_ONES:ROW_ONES + 1, :],
                                  in_=xT[0:1, g * Bg:(g + 1) * Bg])
                # x block 0 (steps 0..3) into rows 116..119
                nc.sync.dma_start(out=stk[ROW_X:ROW_X + XBLK, :],
                                  in_=xT[1:1 + XBLK, g * Bg:(g + 1) * Bg])
                stks.append(stk)
                cts.append(ct)
                gps.append(gp)
                pos.append(opsum.tile([128, nchunk * C], fp, tag=f"po{g}",
                                      name=f"po{g}"))
            warm = opsum.tile([128, 8], fp, tag="warm", name="warm")

            mult = mybir.AluOpType.mult
            add = mybir.AluOpType.add
            tanh = mybir.ActivationFunctionType.Tanh
            sigm = mybir.ActivationFunctionType.Sigmoid

            for q in range(T + 1):
                mm = q <= T
                var = q % NVAR
                # ---- x prefetch: at the start of block k, fetch block k+1
                # into the other x-row half (its readers finished 2 blocks
                # ago, so the DMA is fully off the critical path).
                for g in range(groups):
                    stk, gp = stks[g], gps[g]
                    cols = slice(g * Bg, (g + 1) * Bg)
                    if q % XBLK == 0 and q + XBLK < T:
                        k1 = q // XBLK + 1
                        r0 = ROW_X + (k1 % 2) * XBLK
                        nc.sync.dma_start(
                            out=stk[r0:r0 + XBLK, :],
                            in_=xT[1 + k1 * XBLK:1 + (k1 + 1) * XBLK, cols])
                    if mm:
                        rhs = stk[0:K_STK, :]
                        for xi in range(4):
                            cb = (var * 4 + xi) * MW
                            nc.tensor.matmul(
                                gp[0:GP, xi * Bg:(xi + 1) * Bg],
                                wg[0:K_STK, cb:cb + MW],
                                rhs, start=True, stop=True)
                # ---- elementwise chain per group. Banks: 0=f, 1=i, 2=g, 3=o.
                # All gates through ONE fused tanh (minimal ACT op count).
                if mm:
                    for g in range(groups):
                        sg_t = temps.tile([GP, 4 * Bg], dt_e, tag=f"sg{g}")
                        nc.scalar.activation(sg_t, gps[g][0:GP, :], tanh)
                        tf = sg_t[:, 0 * Bg:1 * Bg]
                        ti = sg_t[:, 1 * Bg:2 * Bg]
                        tg = sg_t[:, 2 * Bg:3 * Bg]
                        to = sg_t[:, 3 * Bg:4 * Bg]
                        v = temps.tile([GP, Bg], dt_e, tag=f"v{g}")
                        u = temps.tile([GP, Bg], dt_e, tag=f"u{g}")
                        tcl = temps.tile([GP, Bg], dt_e, tag=f"tc{g}")
                        # u = (tf+1)*ct ; v = (ti+1)*tg ; ct = 0.5*u + v
                        nc.vector.scalar_tensor_tensor(u, tf, 1.0, cts[g][:, :],
                                                       add, mult)
                        nc.vector.scalar_tensor_tensor(v, ti, 1.0, tg,
                                                       add, mult)
                        nc.vector.scalar_tensor_tensor(cts[g][:, :], u, 0.5,
                                                       v, mult, add)
                        # PE keep-warm: a ~7ns f32 N=1 matmul anchored on
                        # sg_t runs mid-phase, so the PE never idles long
                        # enough for the cost model to drop it out of the
                        # fast pstate (which would turn the next phase's
                        # 107ns gate matmuls into 394ns ones).
                        sgf = sg_t.bitcast(fp)
                        nc.tensor.matmul(warm[:, g:g + 1], sgf[0:32, 0:128],
                                         sgf[0:32, 0:1], start=True, stop=True)
                        # tc = tanh(0.5*ct) ; ht = (to+1)*tc (into f32r stk)
                        nc.scalar.activation(tcl, cts[g][:, :], tanh, scale=0.5)
                        nc.vector.scalar_tensor_tensor(
                            stks[g][ROW_H1:ROW_H1 + GP, :], to, 1.0, tcl,
                            add, mult)
                        tcf = tcl.bitcast(fp)
                        nc.tensor.matmul(warm[0:64, 4 + g:5 + g],
                                         tcf[0:32, 0:64], tcf[0:32, 0:1],
                                         start=True, stop=True)
                        # ---- out head for step t = q-1 (reads the h2 rows
                        # just written by ht: h2(q-1)). Emitted here so the
                        # head matmuls run mid-phase, which also keeps the PE
                        # from idling long enough to drop out of its fast
                        # pstate.
                        if q >= 1:
                            t = q - 1
                            tc_col = t % C
                            hc = NVAR * 4 * MW
                            for k in range(nchunk):
                                # f32r rejects N=1 matmuls; f32 bitcast views
                                nc.tensor.matmul(
                                    pos[g][:, k * C + tc_col:k * C + tc_col + 1],
                                    stks[g][64:116, k * 128:(k + 1) * 128]
                                    .bitcast(fp),
                                    wg[64:116, hc:hc + 1].bitcast(fp),
                                    start=True, stop=True)
                            if tc_col == C - 1:  # flush epoch
                                t0 = t - (C - 1)
                                for k in range(nchunk):
                                    st = ostage.tile([128, C], fp,
                                                     tag=f"os{g}_{k}",
                                                     name=f"os{g}_{k}")
                                    nc.vector.tensor_copy(
                                        st, pos[g][:, k * C:(k + 1) * C])
                                    row0 = g * Bg + k * 128
                                    nc.sync.dma_start(
                                        out=out_bt[row0:row0 + 128, t0:t0 + C],
                                        in_=st)
                if q == 0:
                    # cancel phase 0's spurious l2 output: h2/c2 must enter
                    # phase 1 as zero.
                    for g in range(groups):
                        nc.vector.memset(
                            stks[g][ROW_H2:ROW_H2 + H, :].bitcast(fp), 0.0)
                        nc.vector.memset(
                            cts[g][ROW_H2:ROW_H2 + H, :].bitcast(fp), 0.0)
    nc.compile()
    return nc


_NC_CACHE = {}


def _get_nc(T, B, groups, mode):
    key = (T, B, groups, mode)
    if key not in _NC_CACHE:
        _NC_CACHE[key] = build_core_kernel(T, B, groups, mode)
    return _NC_CACHE[key]


def kernel(input, W_ih1, W_hh1, b_ih1, b_hh1, W_ih2, W_hh2, b_ih2, b_hh2,
           W_lin, b_lin, _groups=2, _mode="bf16"):
    from concourse import bass_utils

    input = np.asarray(input, dtype=np.float32)
    B, T = input.shape
    Bc = B // N_CORES
    WG = _build_weights(np.asarray(W_ih1, np.float64), np.asarray(W_hh1, np.float64),
                        np.asarray(b_ih1, np.float64), np.asarray(b_hh1, np.float64),
                        np.asarray(W_ih2, np.float64), np.asarray(W_hh2, np.float64),
                        np.asarray(b_ih2, np.float64), np.asarray(b_hh2, np.float64),
                        np.asarray(W_lin, np.float64), np.asarray(b_lin, np.float64))
    # row 0 = ones (bias row), rows 1..T = input.T
    xT = np.concatenate([np.ones((1, B), np.float32),
                         input.T.astype(np.float32)])
    WGh = np.ascontiguousarray(WG).astype(np.float32)
    nc = _get_nc(T, Bc, _groups, _mode)
    in_maps = [
        {"xT": np.ascontiguousarray(xT[:, c * Bc:(c + 1) * Bc]), "WG": WGh}
        for c in range(N_CORES)
    ]
    res = bass_utils.run_bass_kernel_spmd(
        nc, in_maps, core_ids=list(range(N_CORES)), trace=False)
    outs = [res.results[c]["out_bt"] for c in range(N_CORES)]  # [Bc, T] each
    out = np.concatenate(outs, axis=0)  # [B, T]
    return out.astype(np.float32)
